# revision 1
# baseline (speedup 1.0000x reference)
"""Trainium2 Bass kernel for nn_ANO_VQC_Model (14-qubit VQC, batch 512).

Math: the circuit state, viewed as a 128x128 matrix M (rows = qubits 0-6,
cols = qubits 7-13), starts as a real rank-1 outer product u v^T and each
entangling layer k acts as M' = A_k CNOT67(M) B_k^T with A_k, B_k pure
orthogonal local operators and CNOT67(M) = E0 M + E1 M F (E0/E1 = projectors
on qubit 6 = row LSB, F = X on qubit 7 = column-half swap).  So the state
stays factored L R^T with L <- A[E0 L | E1 L], R <- B[R | F R]; rank doubles
per layer.

Only 5 layers are simulated (rank 32).  The 6th layer is folded into the
measurement: with M6 = A6 N B6^T, N = CNOT67(M5), orthogonality of B6 gives
    e_q = <T_dq, S> + <T_oq, S'>,
    T_dq = L^T Hd_q L, T_oq = L^T Ho_q L, S = R^T R, S' = R^T F R,
where Hd_q/Ho_q are the qubit-6 block-diagonal/off-diagonal parts of
A6^T Re(H_q) A6.  A host-side orthogonal rotation Q diagonalizes Hd_0
(folded into the stage-2 A matrices), so T_d0 needs only a per-partition
scale of L.  Everything is real f32 math done in f16 on the PE.

Device schedule: stage1 = layers 0-2 (8 terms, rank 8), stage2 = layers 3-4
(4 terms, rank 32), K = the three dense measurement-matrix products,
per-sample Gram matmuls with M=32 column-tiling (4 samples concurrently on
the PE array), elementwise product + reduction.

Sharding: pure data parallel, 64 batch elements per core on 8 cores.
"""

import os
import sys

import numpy as np

for _p in ("/opt/trn_rl_repo", "/root/.axon_site/_ro/trn_rl_repo"):
    if os.path.isdir(_p) and _p not in sys.path:
        sys.path.append(_p)

import concourse.bass as bass
import concourse.mybir as mybir
import concourse.tile as tile
from concourse import bacc
from concourse.bass_utils import run_bass_kernel_spmd


def _ensure_ntff_hook():
    """bass_utils imports antenv.axon_hooks when tracing; some images lack
    it.  Provide a shim (and register the ctypes NTFF hook when possible)."""
    try:
        import antenv.axon_hooks  # noqa: F401

        return
    except ImportError:
        pass
    try:
        import types

        import antenv

        mod = types.ModuleType("antenv.axon_hooks")
        holder = {}
        mod.set_axon_ntff_profile_hook = lambda h: holder.__setitem__("h", h)
        mod.get_axon_ntff_profile_hook = lambda: holder.get("h")
        sys.modules["antenv.axon_hooks"] = mod
        antenv.axon_hooks = mod
        try:
            from trn_agent_boot.trn_boot import _ntff_profile_via_ctypes

            hook = _ntff_profile_via_ctypes("/opt/axon/libaxon_pjrt.so")
            if hook is not None:
                mod.set_axon_ntff_profile_hook(hook)
        except Exception:
            pass
    except Exception:
        pass


_ensure_ntff_hook()

N_CORES = 8
BATCH = 512
BPC = BATCH // N_CORES  # 64
DEPTH = 6
DA = 128
DB = 128

F32 = mybir.dt.float32
MM_DT = mybir.dt.float16

_nc_cache = {}


# ----------------------------------------------------------------------------
# Host-side preprocessing (input-dependent constant folding)
# ----------------------------------------------------------------------------

def _ry(t):
    c, s = np.cos(t / 2), np.sin(t / 2)
    return np.array([[c, -s], [s, c]], dtype=np.float64)


_CNOT = np.array(
    [[1, 0, 0, 0], [0, 1, 0, 0], [0, 0, 0, 1], [0, 0, 1, 0]], dtype=np.float64
)


def _kron_list(ms):
    out = ms[0]
    for m in ms[1:]:
        out = np.kron(out, m)
    return out


def _cnot_on(n, ctrl):
    mats, q = [], 0
    while q < n:
        if q == ctrl:
            mats.append(_CNOT)
            q += 2
        else:
            mats.append(np.eye(2))
            q += 1
    return _kron_list(mats)


def _layer_ops(theta_k):
    """Pure-orthogonal (A, B) for one layer; CNOT67 handled separately."""
    C_evenA = _cnot_on(7, 0) @ _cnot_on(7, 2) @ _cnot_on(7, 4)
    C_oddA = _cnot_on(7, 1) @ _cnot_on(7, 3) @ _cnot_on(7, 5)
    R_A = _kron_list([_ry(theta_k[w]) for w in range(7)])
    C_evenB = _cnot_on(7, 1) @ _cnot_on(7, 3) @ _cnot_on(7, 5)
    C_oddB = _cnot_on(7, 0) @ _cnot_on(7, 2) @ _cnot_on(7, 4)
    R_B = _kron_list([_ry(theta_k[7 + w]) for w in range(7)])
    return R_A @ C_oddA @ C_evenA, R_B @ C_oddB @ C_evenB


def _measure_mats(Ain, Din):
    """G_q = Re(H_q) expanded on the 128-dim row space, q = 0, 1."""
    NLOC = 8
    r, c = np.tril_indices(NLOC, -1)
    Gs = []
    for q in range(2):
        tri = np.zeros((NLOC, NLOC))
        tri[r, c] = Ain[q]
        h = tri + np.diag(np.concatenate([Din[q][1:], [0.0]]))
        Hr = h + h.T
        if q == 0:
            Gs.append(np.kron(Hr, np.eye(16)))
        else:
            Gs.append(np.kron(np.kron(np.eye(2), Hr), np.eye(8)))
    return Gs


def _host_prep(X, theta, Ain, Bin, Din):
    X = np.asarray(X, dtype=np.float64)
    theta = np.asarray(theta, dtype=np.float64)
    nb = X.shape[0]
    c, s = np.cos(X / 2), np.sin(X / 2)
    v0 = (c - s) / np.sqrt(2.0)
    v1 = (c + s) / np.sqrt(2.0)

    def kron_side(ws):
        out = np.ones((nb, 1))
        for w in ws:
            pair = np.stack([v0[:, w], v1[:, w]], axis=1)
            out = (out[:, :, None] * pair[:, None, :]).reshape(nb, -1)
        return out

    U = kron_side(range(7))       # (B, 128), qubit 0 = MSB ... qubit 6 = LSB
    V = kron_side(range(7, 14))   # (B, 128), qubit 7 = MSB

    As, Bs = zip(*[_layer_ops(theta[k]) for k in range(DEPTH)])
    rows = np.arange(DA)
    e0 = (rows % 2 == 0).astype(np.float64)
    E = [np.diag(e0), np.diag(1.0 - e0)]
    F = np.zeros((DB, DB))
    F[:64, 64:] = np.eye(64)
    F[64:, :64] = np.eye(64)

    # measurement: fold layer 6, rotate rows by Q diagonalizing Hd_0
    G0, G1 = _measure_mats(Ain, Din)
    A6 = As[5]
    Hds, Hos = [], []
    for G in (G0, G1):
        Ht = A6.T @ G @ A6
        Hds.append(E[0] @ Ht @ E[0] + E[1] @ Ht @ E[1])
        Hos.append(E[0] @ Ht @ E[1] + E[1] @ Ht @ E[0])
    # Hd_0 is block-diagonal w.r.t. even/odd rows only after a parity
    # permutation; just eigendecompose the full (symmetric) matrix.
    mu, Q = np.linalg.eigh(Hds[0])
    hk = np.stack(
        [Q.T @ Hos[0] @ Q, Q.T @ Hds[1] @ Q, Q.T @ Hos[1] @ Q], axis=0
    )  # (3, 128, 128) symmetric

    # stage 1: layers 0-1, 4 terms; c2 = p1*2 + p0
    F1A = np.empty((4, DA, DA))
    F1B = np.empty((4, DB, DB))
    for cw in range(4):
        p0, p1 = cw & 1, (cw >> 1) & 1
        F1A[cw] = As[1] @ E[p1] @ As[0] @ E[p0]
        F1B[cw] = (
            Bs[1] @ np.linalg.matrix_power(F, p1)
            @ Bs[0] @ np.linalg.matrix_power(F, p0)
        )
    # stage 2: layers 2-4, 8 terms; a = p4*4 + p3*2 + p2; Q^T folded into A
    F2A = np.empty((8, DA, DA))
    F2B = np.empty((8, DB, DB))
    for aw_ in range(8):
        p2, p3, p4 = aw_ & 1, (aw_ >> 1) & 1, (aw_ >> 2) & 1
        F2A[aw_] = Q.T @ As[4] @ E[p4] @ As[3] @ E[p3] @ As[2] @ E[p2]
        F2B[aw_] = (
            Bs[4] @ np.linalg.matrix_power(F, p4)
            @ Bs[3] @ np.linalg.matrix_power(F, p3)
            @ Bs[2] @ np.linalg.matrix_power(F, p2)
        )

    # lhsT packs (out = lhsT.T @ rhs -> store transposed)
    wa1 = np.concatenate([F1A[cw].T for cw in range(4)], axis=1)  # (128, 512)
    wb1 = np.concatenate([F1B[cw].T for cw in range(4)], axis=1)
    wa2 = np.concatenate([F2A[aw_].T for aw_ in range(8)], axis=1)  # (128, 1024)
    wb2 = np.concatenate([F2B[aw_].T for aw_ in range(8)], axis=1)
    whk = np.concatenate([hk[i] for i in range(3)], axis=1)  # (128, 384), sym
    return U, V, wa1, wb1, wa2, wb2, whk, mu


# ----------------------------------------------------------------------------
# Device kernel
# ----------------------------------------------------------------------------

def _build_nc():
    nc = bacc.Bacc("TRN2", target_bir_lowering=False, debug=False)

    ut_d = nc.declare_dram_parameter("ut", [DA, BPC], MM_DT, isOutput=False)
    vt_d = nc.declare_dram_parameter("vt", [DB, BPC], MM_DT, isOutput=False)
    wa1_d = nc.declare_dram_parameter("wa1", [DA, 512], MM_DT, isOutput=False)
    wb1_d = nc.declare_dram_parameter("wb1", [DB, 512], MM_DT, isOutput=False)
    wa2_d = nc.declare_dram_parameter("wa2", [DA, 1024], MM_DT, isOutput=False)
    wb2_d = nc.declare_dram_parameter("wb2", [DB, 1024], MM_DT, isOutput=False)
    whk_d = nc.declare_dram_parameter("whk", [DA, 384], MM_DT, isOutput=False)
    mu_d = nc.declare_dram_parameter("mu", [DA, 1], F32, isOutput=False)
    out_d = nc.declare_dram_parameter("out", [4, 32], F32, isOutput=True)

    with tile.TileContext(nc) as tc:
        with (
            tc.tile_pool(name="w", bufs=1) as wpool,
            tc.tile_pool(name="state", bufs=1) as spool,
            tc.tile_pool(name="ps", bufs=3, space="PSUM") as pbig,   # 2 banks each
            tc.tile_pool(name="ps2", bufs=2, space="PSUM") as psmall,  # 1 bank each
        ):
            aw1 = wpool.tile([DA, 512], MM_DT, tag="aw1")
            bw1 = wpool.tile([DB, 512], MM_DT, tag="bw1")
            aw2 = wpool.tile([DA, 1024], MM_DT, tag="aw2")
            bw2 = wpool.tile([DB, 1024], MM_DT, tag="bw2")
            hkw = wpool.tile([DA, 384], MM_DT, tag="hkw")
            ut = wpool.tile([DA, BPC], MM_DT, tag="ut")
            vt = wpool.tile([DB, BPC], MM_DT, tag="vt")
            muT = wpool.tile([DA, 1], F32, tag="mu")
            sel = wpool.tile([128, 4], MM_DT, tag="sel")
            warm = wpool.tile([128, 512], MM_DT, tag="warm")

            # input DMAs spread over three queues, just-in-time order
            nc.sync.dma_start(out=ut[:], in_=ut_d[:, :])
            nc.sync.dma_start(out=aw1[:], in_=wa1_d[:, :])
            nc.gpsimd.dma_start(out=vt[:], in_=vt_d[:, :])
            nc.gpsimd.dma_start(out=bw1[:], in_=wb1_d[:, :])
            nc.scalar.dma_start(out=aw2[:], in_=wa2_d[:, :])
            nc.sync.dma_start(out=bw2[:, 0:512], in_=wb2_d[:, 0:512])
            nc.gpsimd.dma_start(out=bw2[:, 512:1024], in_=wb2_d[:, 512:1024])
            nc.sync.dma_start(out=hkw[:], in_=whk_d[:, :])
            nc.gpsimd.dma_start(out=muT[:], in_=mu_d[:, :])

            nc.vector.memset(warm[:], 0.125)
            nc.vector.memset(sel[:], 0.0)
            for m in range(4):
                nc.vector.memset(sel[32 * m:32 * m + 32, m:m + 1], 1.0)

            # warmup: back-to-back dummy matmuls during the input DMA window
            # flip the HAM clock gate to 8/8 before real compute starts
            for _ in range(4):
                wps = pbig.tile([128, 1024], F32, tag="big")
                nc.tensor.matmul(
                    wps[:, 0:512], warm[:, 0:128], warm[:], start=True, stop=True
                )
                nc.tensor.matmul(
                    wps[:, 512:1024], warm[:, 0:128], warm[:], start=True, stop=True
                )

            L3 = spool.tile([DA, 256], MM_DT, tag="L3")
            R3 = spool.tile([DB, 256], MM_DT, tag="R3")
            Lb = spool.tile([DA, 2048], MM_DT, tag="Lb")
            RF = spool.tile([DB, 4096], MM_DT, tag="RF")  # cols 0:2048 R, 2048: FR
            Pb = spool.tile([DA, 8192], MM_DT, tag="Pb")
            SS = spool.tile([128, 1024], MM_DT, tag="SS")
            tbT = spool.tile([128, 1024], MM_DT, tag="tbT")
            tb = spool.tile([128, 2048], MM_DT, tag="tb")
            esb = spool.tile([4, 32], F32, tag="esb")

            # ---- stage 1: 4 terms per side, N=64, c-major output ------------
            s1 = pbig.tile([128, 1024], F32, tag="big")
            for cw in range(4):
                nc.tensor.matmul(
                    s1[:, cw * 64:(cw + 1) * 64],
                    aw1[:, cw * 128:(cw + 1) * 128], ut[:],
                    start=True, stop=True,
                )
            for cw in range(4):
                nc.tensor.matmul(
                    s1[:, 512 + cw * 64:512 + (cw + 1) * 64],
                    bw1[:, cw * 128:(cw + 1) * 128], vt[:],
                    start=True, stop=True,
                )
            nc.vector.tensor_copy(L3[:], s1[:, 0:256])
            nc.scalar.copy(out=R3[:], in_=s1[:, 512:768])

            # ---- stage 2: 8 terms per side, b-major rhs reads ---------------
            # rhs = L2 read as (b, c2); psum cols (b, c2); evict to
            # Lb cols b*32 + a*4 + c2  /  RF cols b*32 + a*4 + c2
            L3v = L3[:].rearrange("p (c b) -> p b c", c=4)
            R3v = R3[:].rearrange("p (c b) -> p b c", c=4)
            Lbv = Lb[:].rearrange("p (b a c) -> p a b c", a=8, c=4)
            Rbv = RF[:, 0:2048].rearrange("p (b a c) -> p a b c", a=8, c=4)

            def stage2(w_tile, rhs_v, dst_v):
                for half in range(2):
                    s2 = pbig.tile([128, 1024], F32, tag="big")
                    for i in range(4):
                        nc.tensor.matmul(
                            s2[:, i * 256:(i + 1) * 256],
                            w_tile[:, (4 * half + i) * 128:(4 * half + i + 1) * 128],
                            rhs_v, start=True, stop=True,
                        )
                    s2v = s2[:].rearrange("p (a b c) -> p a b c", a=4, c=4)
                    dv = dst_v[:, 4 * half:4 * half + 4]
                    # bank-parallel eviction: DVE takes bank 0, ACT bank 1
                    nc.vector.tensor_copy(dv[:, 0:2], s2v[:, 0:2])
                    nc.scalar.copy(out=dv[:, 2:4], in_=s2v[:, 2:4])

            stage2(aw2, L3v, Lbv)
            stage2(bw2, R3v, Rbv)

            # ---- F R: swap partition halves (two contiguous SBUF DMAs) ------
            nc.sync.dma_start(out=RF[0:64, 2048:4096], in_=RF[64:128, 0:2048])
            nc.sync.dma_start(out=RF[64:128, 2048:4096], in_=RF[0:64, 0:2048])

            # ---- P buffer: [mu*L | Ho0 L | Hd1 L | Ho1 L] per sample --------
            Pv = Pb[:].rearrange("p (b s i) -> p s b i", s=4, i=32)
            nc.vector.tensor_scalar_mul(
                Pv[:, 0], Lb[:].rearrange("p (b i) -> p b i", i=32), muT[:]
            )

            # K matmuls read Lb in a-major chunks so each chunk only depends
            # on a single stage-2 eviction op
            Lbk = Lb[:].rearrange("p (b a c) -> p b a c", a=8, c=4)
            Pk = Pb[:].rearrange("p (b s a c) -> p s b a c", s=4, a=8, c=4)

            def k_mat(im):
                for half in range(2):
                    kp = pbig.tile([128, 1024], F32, tag="big")
                    for i in range(2):
                        j = 2 * half + i
                        nc.tensor.matmul(
                            kp[:, i * 512:(i + 1) * 512],
                            hkw[:, im * 128:(im + 1) * 128],
                            Lbk[:, :, 2 * j:2 * j + 2, :],
                            start=True, stop=True,
                        )
                    kv = kp[:].rearrange("p (i b a c) -> p i b a c", i=2, a=2, c=4)
                    dst = Pk[:, im + 1]
                    nc.vector.tensor_copy(
                        dst[:, :, 4 * half:4 * half + 2, :], kv[:, 0]
                    )
                    nc.scalar.copy(
                        out=dst[:, :, 4 * half + 2:4 * half + 4, :], in_=kv[:, 1]
                    )

            # per-sample S-Grams: rhs = [R_b | FR_b] via 2-dim AP over RF
            RFs = RF[:].rearrange("p (h b i) -> p b h i", h=2, i=32)

            def gram_s(g2):
                sp = psmall.tile([128, 512], F32, tag="small")
                for g in range(8 * g2, 8 * g2 + 8):
                    for k in range(4):
                        b = g * 4 + k
                        nc.tensor.matmul(
                            sp[32 * k:32 * k + 32,
                               (g - 8 * g2) * 64:(g - 8 * g2 + 1) * 64],
                            RF[:, b * 32:(b + 1) * 32],
                            RFs[:, b],
                            start=True, stop=True, tile_position=(0, 32 * k),
                        )
                nc.scalar.copy(out=SS[:, g2 * 512:(g2 + 1) * 512], in_=sp[:])

            # interleave K-matrix matmuls with S-Grams to keep the PE dense
            k_mat(0)
            gram_s(0)
            k_mat(1)
            gram_s(1)
            k_mat(2)

            # ---- T-Grams + elementwise product ------------------------------
            for g2 in range(2):
                tp = pbig.tile([128, 1024], F32, tag="big")
                for g in range(8 * g2, 8 * g2 + 8):
                    for k in range(4):
                        b = g * 4 + k
                        nc.tensor.matmul(
                            tp[32 * k:32 * k + 32,
                               (g - 8 * g2) * 128:(g - 8 * g2 + 1) * 128],
                            Lb[:, b * 32:(b + 1) * 32],
                            Pb[:, b * 128:(b + 1) * 128],
                            start=True, stop=True, tile_position=(0, 32 * k),
                        )
                ssv = SS[:, g2 * 512:(g2 + 1) * 512].rearrange(
                    "p (g j) -> p g j", g=8
                ).unsqueeze(2).broadcast_to((128, 8, 2, 64))
                tbv = tb[:, g2 * 1024:(g2 + 1) * 1024].rearrange(
                    "p (g q j) -> p g q j", g=8, q=2, j=64
                )
                if g2 == 0:
                    # offload: ACT evicts, GpSimd multiplies (SBUF x SBUF)
                    nc.scalar.copy(out=tbT[:], in_=tp[:])
                    nc.gpsimd.tensor_mul(
                        tbv,
                        tbT[:].rearrange("p (g q j) -> p g q j", g=8, q=2, j=64),
                        ssv,
                    )
                else:
                    tpv = tp[:].rearrange("p (g q j) -> p g q j", g=8, q=2, j=64)
                    nc.vector.tensor_mul(tbv, tpv, ssv)

            # ---- reduce: e_q[g*4+k] = sum over partition block k, 64 cols ---
            tbr = tb[:].rearrange("p (g q c j) -> p q c g j", g=16, q=2, c=4, j=16)
            for q in range(2):
                zp = psmall.tile([4, 256], F32, tag="small")
                for c in range(4):
                    nc.tensor.matmul(
                        zp[:], sel[:], tbr[:, q, c],
                        start=(c == 0), stop=(c == 3),
                    )
                nc.vector.reduce_sum(
                    out=esb[:, q * 16:(q + 1) * 16],
                    in_=zp[:].rearrange("p (g j) -> p g j", j=16),
                    axis=mybir.AxisListType.X,
                )

            nc.sync.dma_start(out=out_d[:, :], in_=esb[:])

    nc.compile()
    return nc


def _get_nc():
    if "nc" not in _nc_cache:
        _nc_cache["nc"] = _build_nc()
    return _nc_cache["nc"]


# ----------------------------------------------------------------------------
# Entry point
# ----------------------------------------------------------------------------

def kernel(X, theta, A, B, D, _trace=False):
    U, V, wa1, wb1, wa2, wb2, whk, mu = _host_prep(X, theta, A, B, D)
    np_mm = mybir.dt.np(MM_DT)
    wa1 = np.ascontiguousarray(wa1, dtype=np_mm)
    wb1 = np.ascontiguousarray(wb1, dtype=np_mm)
    wa2 = np.ascontiguousarray(wa2, dtype=np_mm)
    wb2 = np.ascontiguousarray(wb2, dtype=np_mm)
    whk = np.ascontiguousarray(whk, dtype=np_mm)
    mu_a = np.ascontiguousarray(mu.reshape(DA, 1), dtype=np.float32)
    in_maps = []
    for i in range(N_CORES):
        sl = slice(i * BPC, (i + 1) * BPC)
        in_maps.append(
            {
                "ut": np.ascontiguousarray(U[sl].T, dtype=np_mm),
                "vt": np.ascontiguousarray(V[sl].T, dtype=np_mm),
                "wa1": wa1, "wb1": wb1, "wa2": wa2, "wb2": wb2,
                "whk": whk, "mu": mu_a,
            }
        )
    nc = _get_nc()
    kw = {}
    if _trace:
        import shutil
        import tempfile

        shutil.rmtree("/tmp/vqc_prof", ignore_errors=True)
        os.makedirs("/tmp/vqc_prof", exist_ok=True)
        kw["tmpdir"] = tempfile.mkdtemp(dir="/tmp/vqc_prof")
    res = run_bass_kernel_spmd(nc, in_maps, list(range(N_CORES)), trace=_trace, **kw)
    outs = []
    for i in range(N_CORES):
        raw = res.results[i]["out"]  # (4, 32): [k, q*16 + g]
        e = np.empty((BPC, 2), dtype=np.float32)
        for q in range(2):
            for g in range(16):
                for k in range(4):
                    e[g * 4 + k, q] = raw[k, q * 16 + g]
        outs.append(e)
    full = np.concatenate(outs, axis=0).astype(np.float32)
    if _trace:
        _nc_cache["last_exec_ns"] = res.exec_time_ns
        _nc_cache["last_results"] = res
    return full

